# revision 28
# baseline (speedup 1.0000x reference)
"""Trainium2 Bass kernel for a 7-layer ternary-weight (BitNet) 1D conv
feature extractor with exact-erf GELU after each layer.

Contract: kernel(**inputs) takes the FULL inputs from setup_inputs()
(x: [8, 160000] f32, w0..w6 / b0..b6 conv params) and returns the full
output [8, 256, 500] f32.

v2 design (data-parallel, one batch element per core):
- L0 (Cin=1, K=10, stride 5) runs 4x row-tiled (tile_position=(32r,0),
  contraction 10): four 512-col output slices stream concurrently, so a
  2048-col output group costs ~512 PE cycles. Host preps a phase buffer
  xr4[10r+j, 512g+w] = xpad[5*(2048g+512r+w)+j].
- cout=192 layers (L1..L3) store channels 0..127 in a plain tile t1
  [128, 2, H+4] (dim1 = sequence half, one halo col) and channels
  128..191 in a packed tile t2 [128, H+1]: rows 0:64 = first half of
  the sequence, rows 64:128 = second half. t2 is produced with 2x
  column-tiled matmuls (both halves concurrently, M=64 each) and
  consumed with 2x row-tiled K=64 matmuls (both halves concurrently),
  keeping the PE array fully packed where v1 wasted half of it, and
  making every GELU a full 128-partition activation.
- Activations are [128, ~2048] (one 4-bank PSUM slot, 2-slot rotation)
  to amortize the ~293ns/instr ACT overhead.
- Emission interleaves L0 groups with L1 units (the ACT-bound phase) and
  streams L2..L6 as soon as their inputs are emitted.
"""

import numpy as np

# (in_ch, out_ch, kernel, stride, pad) -- fixed problem geometry
LAYERS = [(1, 128, 10, 5, 4), (128, 192, 3, 2, 1), (192, 192, 3, 2, 1),
          (192, 192, 3, 2, 1), (192, 256, 3, 2, 1), (256, 256, 4, 2, 1),
          (256, 256, 4, 2, 1)]
T_IN = 160000
LOUT = [32000, 16000, 8000, 4000, 2000, 1000, 500]
N_CORES = 8
NT = 512          # max matmul free dim / one fp32 PSUM bank
GW = 2048         # PSUM slot width (4 banks)
H = {i: LOUT[i] // 2 for i in (1, 2, 3)}  # packed-tile half lengths


def _slices(n, w=NT):
    return [(o, min(w, n - o)) for o in range(0, n, w)]


def _pairs(i):
    """c1-joint pair supertiles (t0, nst) over [0, H[i])."""
    return _slices(H[i], 1024)


# ---------------- host-side packing ----------------

def _wlayout():
    cols = {}
    tot = 0

    def add(key, n):
        nonlocal tot
        cols[key] = tot
        tot += n

    add((0,), 128)
    add((1, 'c1k128'), 3 * 128)
    add((1, 'c2k128'), 3 * 64)
    for i in (2, 3):
        add((i, 'c1k128'), 3 * 128)
        add((i, 'c1k64'), 3 * 128)
        add((i, 'c2k128'), 3 * 64)
        add((i, 'c2k64'), 3 * 64)
    add((4, 'c1k128'), 3 * 256)   # per tap: [mi=0 | mi=1]
    add((4, 'c1k64'), 3 * 256)
    for i in (5, 6):
        add((i, 'm'), 2 * 4 * 2 * 128)  # ti, k, mi major->minor
    return cols, tot


WCOLS, WTOT = _wlayout()
BCOLS = {}
_nb = 0
for _i in range(7):
    for _mi in range(2 if LAYERS[_i][1] > 128 else 1):
        BCOLS[(_i, _mi)] = _nb
        _nb += 2
NB = _nb


def _pack_host(ws, bs):
    wpk = np.zeros((128, WTOT), np.float16)
    bpk = np.zeros((128, NB), np.float32)
    signs = []
    for i in range(7):
        w = np.asarray(ws[i], np.float32)
        scale = max(float(np.mean(np.abs(w))), 1e-5)
        signs.append(np.clip(np.round(w / scale), -1.0, 1.0))
        b = np.asarray(bs[i], np.float32)
        cout = LAYERS[i][1]
        for mi in range(2 if cout > 128 else 1):
            m0 = 128 * mi
            msz = min(128, cout - m0)
            c = BCOLS[(i, mi)]
            bpk[0:msz, c] = b[m0:m0 + msz]
            bpk[0:msz, c + 1] = scale
            if msz == 64:  # packed chunk: duplicate on rows 64:128
                bpk[64:128, c] = b[m0:m0 + 64]
                bpk[64:128, c + 1] = scale
    f16 = np.float16
    s0 = signs[0][:, 0, :]  # [128, 10]
    for r in range(4):
        wpk[32 * r:32 * r + 10, WCOLS[(0,)]:WCOLS[(0,)] + 128] = \
            s0.T.astype(f16)
    for i in (1, 2, 3, 4):
        s = signs[i]
        cin, cout = LAYERS[i][0], LAYERS[i][1]
        nm1 = 2 if i == 4 else 1
        base = WCOLS[(i, 'c1k128')]
        for k in range(3):
            for mi in range(nm1):
                c0 = base + k * 128 * nm1 + mi * 128
                wpk[0:128, c0:c0 + 128] = \
                    s[128 * mi:128 * mi + 128, 0:128, k].T.astype(f16)
        if cin == 192:
            base = WCOLS[(i, 'c1k64')]
            for k in range(3):
                for mi in range(nm1):
                    c0 = base + k * 128 * nm1 + mi * 128
                    blk = s[128 * mi:128 * mi + 128, 128:192, k].T.astype(f16)
                    wpk[0:64, c0:c0 + 128] = blk
                    wpk[64:128, c0:c0 + 128] = blk
        if cout == 192:
            base = WCOLS[(i, 'c2k128')]
            for k in range(3):
                wpk[0:128, base + 64 * k:base + 64 * k + 64] = \
                    s[128:192, 0:128, k].T.astype(f16)
            if cin == 192:
                base = WCOLS[(i, 'c2k64')]
                for k in range(3):
                    blk = s[128:192, 128:192, k].T.astype(f16)
                    wpk[0:64, base + 64 * k:base + 64 * k + 64] = blk
                    wpk[64:128, base + 64 * k:base + 64 * k + 64] = blk
    for i in (5, 6):
        s = signs[i]
        base = WCOLS[(i, 'm')]
        n = 0
        for ti in range(2):
            for k in range(4):
                for mi in range(2):
                    wpk[0:128, base + n:base + n + 128] = \
                        s[128 * mi:128 * mi + 128,
                          128 * ti:128 * ti + 128, k].T.astype(f16)
                    n += 128
    return wpk, bpk


def _prep_x(xb):
    """xr4 [40, 8192]: xr4[10r+j, 512g+w] = xpad[5*(2048g+512r+w)+j]."""
    xpad = np.zeros(T_IN + 20, np.float16)
    xpad[4:4 + T_IN] = xb.astype(np.float16)
    xr = np.lib.stride_tricks.as_strided(
        xpad, shape=(10, LOUT[0]), strides=(2, 10))
    xr4 = np.zeros((40, 8192), np.float16)
    for g in range(16):
        for r in range(4):
            c0 = 2048 * g + 512 * r
            n = min(512, max(0, LOUT[0] - c0))
            if n:
                xr4[10 * r:10 * r + 10, 512 * g:512 * g + n] = \
                    xr[:, c0:c0 + n]
    return xr4


_CACHE = {}


def _build(debug=False):
    if ("nc", debug) in _CACHE:
        return _CACHE[("nc", debug)]
    from concourse import bacc
    import concourse.mybir as mybir
    import concourse.tile as tile

    F16 = mybir.dt.float16
    F32 = mybir.dt.float32
    GELU = mybir.ActivationFunctionType.Gelu

    nc = bacc.Bacc("TRN2")
    xr_d = nc.dram_tensor("xr", [40, 8192], F16, kind="ExternalInput")
    wp_d = nc.dram_tensor("wp", [128, WTOT], F16, kind="ExternalInput")
    bp_d = nc.dram_tensor("bp", [128, NB], F32, kind="ExternalInput")
    y_d = nc.dram_tensor("y", [256, 500], F32, kind="ExternalOutput")
    dbg_d = {}

    with tile.TileContext(nc) as tc:
        pools = []

        def mkpool(name, bufs=1, space="SBUF"):
            p = tc.alloc_tile_pool(name=name, bufs=bufs, space=space)
            pools.append(p)
            return p

        wpool = mkpool("wpool")
        wt = wpool.tile([128, WTOT], F16, name="wt")
        bt = wpool.tile([128, NB], F32, name="bt")
        scratch = wpool.tile([128, 512], F16, name="scratch")

        apool = mkpool("apool")
        xr4 = apool.tile([128, 8192], F16, name="xr4")
        a0 = apool.tile([128, LOUT[0] + 3], F16, name="a0")
        t1 = {i: apool.tile([128, 2, H[i] + 4], F16, name=f"t1_{i}")
              for i in (1, 2, 3)}
        t2 = {i: apool.tile([128, H[i] + 1], F16, name=f"t2_{i}")
              for i in (1, 2, 3)}
        a4 = [apool.tile([128, LOUT[4] + 4], F16, name=f"a4_{mi}")
              for mi in range(2)]
        a5 = [apool.tile([128, LOUT[5] + 4], F16, name=f"a5_{mi}")
              for mi in range(2)]
        stage = apool.tile([128, 1000], F32, name="stage")

        nc.vector.memset(scratch[:, :], 0.0)
        nc.vector.memset(a0[:, 0:1], 0.0)
        nc.vector.memset(a0[:, LOUT[0] + 1:LOUT[0] + 3], 0.0)
        for i in (1, 2, 3):
            nc.vector.memset(t1[i][:, 0:2, 0:1], 0.0)
            nc.vector.memset(t1[i][:, 0:2, H[i] + 1:H[i] + 3], 0.0)
            nc.vector.memset(t2[i][0:64, 0:1], 0.0)
        for t, L in [(a4[0], LOUT[4]), (a4[1], LOUT[4]),
                     (a5[0], LOUT[5]), (a5[1], LOUT[5])]:
            nc.vector.memset(t[:, 0:1], 0.0)
            nc.vector.memset(t[:, L + 1:L + 3], 0.0)

        pspool = tc.alloc_tile_pool(name="pspool", bufs=1, space="PSUM")
        psall = pspool.tile([128, 4, 1024], F32, name="psall")
        _cur = [0]

        def slots(n):
            """Claim n consecutive 1024-col PSUM slots (round-robin over
            4). Tile's region tracking orders reuse after the prior
            evacuation. Returns the slot index."""
            if _cur[0] % 4 + n > 4:
                _cur[0] += 4 - _cur[0] % 4
            base = _cur[0] % 4
            _cur[0] += n
            return base

        def junk_mms(n):
            for _ in range(n):
                nc.tensor.matmul(psall[:, 0, 0:512], scratch[0:128, 0:128],
                                 scratch[:, :], start=True, stop=True)

        _evac = [0]
        _pending = []

        def act(ps_ap, dst_ap, i, mi):
            # Alternate PSUM evacuation: half the units GELU straight
            # from PSUM on the ACT engine; the other half are copied out
            # by the (otherwise idle) DVE -- freeing the PSUM slot
            # without waiting on the ACT queue -- and their GELU runs
            # in-place later, batched at iteration boundaries.
            c = BCOLS[(i, mi)]
            _evac[0] ^= 1
            if _evac[0]:
                nc.scalar.activation(dst_ap, ps_ap, GELU,
                                     bias=bt[:, c:c + 1],
                                     scale=bt[:, c + 1:c + 2])
            else:
                nc.vector.tensor_copy(dst_ap, ps_ap)
                _pending.append((dst_ap, c))

        def flush_acts():
            for dst_ap, c in _pending:
                nc.scalar.activation(dst_ap, dst_ap, GELU,
                                     bias=bt[:, c:c + 1],
                                     scale=bt[:, c + 1:c + 2])
            _pending.clear()

        # ---- L0 unit: output cols [2048g, 2048g+2048) ----
        def u_l0(g):
            sl = slots(2)
            gn = min(GW, LOUT[0] - GW * g)
            for r in range(4):
                n = min(512, max(0, gn - 512 * r))
                if not n:
                    continue
                nc.tensor.matmul(
                    psall[:, sl + r // 2, (r % 2) * 512:(r % 2) * 512 + n],
                    wt[32 * r:32 * r + 10, WCOLS[(0,)]:WCOLS[(0,)] + 128],
                    xr4[32 * r:32 * r + 10, 512 * g:512 * g + n],
                    start=True, stop=True, tile_position=(32 * r, 0))
            if gn == GW:
                act(psall[:, sl:sl + 2, :],
                    a0[:, 1 + GW * g:1 + GW * g + gn], 0, 0)
            else:
                n0 = min(gn, 1024)
                act(psall[:, sl, 0:n0],
                    a0[:, 1 + GW * g:1 + GW * g + n0], 0, 0)
                if gn > 1024:
                    act(psall[:, sl + 1, 0:gn - 1024],
                        a0[:, 1 + GW * g + 1024:1 + GW * g + gn], 0, 0)

        # ---- source AP builders (h = sequence half, local out col t0) --
        def src_l1(h, k, t0, n):
            c = 16000 * h + 2 * t0 + k
            return a0[0:128, c:c + 2 * n:2]

        def src_t1(i):
            def f(h, k, t0, n):
                c = 2 * t0 + k
                return t1[i][0:128, h, c:c + 2 * n:2]
            return f

        # ---- c1-joint unit for layer i in (1,2,3): both halves of the
        # 128-channel chunk for pair p ----
        def u_c1(i, p, src):
            t0, nst = _pairs(i)[p]
            cin = LAYERS[i][0]
            sl = slots(2)
            ps = psall[:, sl:sl + 2, :]
            cols = WCOLS[(i, 'c1k128')]
            for k in range(3):
                lhsT = wt[0:128, cols + 128 * k:cols + 128 * k + 128]
                for h in range(2):
                    for s0, sn in _slices(nst):
                        nc.tensor.matmul(
                            ps[:, h, s0:s0 + sn], lhsT,
                            src(h, k, t0 + s0, sn),
                            start=k == 0, stop=cin == 128 and k == 2,
                            skip_group_check=True)
            if cin == 192:
                cols64 = WCOLS[(i, 'c1k64')]
                t2p = t2[i - 1]
                for k in range(3):
                    c0 = cols64 + 128 * k
                    for s0, sn in _slices(nst):
                        for h in range(2):
                            nc.tensor.matmul(
                                ps[:, h, s0:s0 + sn],
                                wt[64 * h:64 * h + 64, c0:c0 + 128],
                                t2p[64 * h:64 * h + 64,
                                    2 * (t0 + s0) + k:
                                    2 * (t0 + s0) + k + 2 * sn - 1:2],
                                start=False, stop=k == 2,
                                skip_group_check=True,
                                tile_position=(64 * h, 0))
            act(ps[:, 0:2, 0:nst], t1[i][:, 0:2, 1 + t0:1 + t0 + nst], i, 0)

        # ---- packed c2 unit for layer i in (1,2,3): one pair p ----
        def u_c2(i, p, src):
            t0, nst = _pairs(i)[p]
            cin = LAYERS[i][0]
            sl = slots(1)
            ps = psall[:, sl, :]
            ck = WCOLS[(i, 'c2k128')]
            for k in range(3):
                lhsT = wt[0:128, ck + 64 * k:ck + 64 * k + 64]
                for s0, sn in _slices(nst):
                    for h in range(2):
                        nc.tensor.matmul(
                            ps[64 * h:64 * h + 64, s0:s0 + sn],
                            lhsT, src(h, k, t0 + s0, sn),
                            start=k == 0,
                            stop=cin == 128 and k == 2 and h == 1,
                            skip_group_check=True,
                            tile_position=(0, 64 * h))
            if cin == 192:
                ck64 = WCOLS[(i, 'c2k64')]
                t2p = t2[i - 1]
                for k in range(3):
                    for s0, sn in _slices(nst):
                        for h in range(2):
                            nc.tensor.matmul(
                                ps[64 * h:64 * h + 64, s0:s0 + sn],
                                wt[64 * h:64 * h + 64,
                                   ck64 + 64 * k:ck64 + 64 * k + 64],
                                t2p[64 * h:64 * h + 64,
                                    2 * (t0 + s0) + k:
                                    2 * (t0 + s0) + k + 2 * sn - 1:2],
                                start=False,
                                stop=k == 2 and h == 1,
                                skip_group_check=True,
                                tile_position=(64 * h, 64 * h))
            act(ps[:, 0:nst], t2[i][:, 1 + t0:1 + t0 + nst], i, 1)

        # ---- L4 unit (chunk mi): 2000 out cols, halves of 1000 ----
        def u_l4(mi):
            sl = slots(2)
            ps = psall[:, sl:sl + 2, :]
            cols = WCOLS[(4, 'c1k128')]
            cols64 = WCOLS[(4, 'c1k64')]
            for k in range(3):
                lhsT = wt[0:128, cols + k * 256 + 128 * mi:
                          cols + k * 256 + 128 * mi + 128]
                for h in range(2):
                    for s0, sn in _slices(1000):
                        nc.tensor.matmul(
                            ps[:, h, s0:s0 + sn], lhsT,
                            src_t1(3)(h, k, s0, sn),
                            start=k == 0, stop=False,
                            skip_group_check=True)
            for k in range(3):
                c0 = cols64 + k * 256 + 128 * mi
                for s0, sn in _slices(1000):
                    for h in range(2):
                        nc.tensor.matmul(
                            ps[:, h, s0:s0 + sn],
                            wt[64 * h:64 * h + 64, c0:c0 + 128],
                            t2[3][64 * h:64 * h + 64,
                                  2 * s0 + k:2 * s0 + k + 2 * sn - 1:2],
                            start=False, stop=k == 2,
                            skip_group_check=True,
                            tile_position=(64 * h, 0))
            act(ps[:, 0, 0:1000], a4[mi][:, 1:1001], 4, mi)
            act(ps[:, 1, 0:1000], a4[mi][:, 1001:2001], 4, mi)

        # ---- L5/L6 plain units ----
        def u_tail(i, mi, srcs, dst):
            lout = LOUT[i]
            sl = slots(1)
            ps = psall[:, sl, :]
            base = WCOLS[(i, 'm')]
            for ti in range(2):
                for k in range(4):
                    c0 = base + (ti * 8 + k * 2 + mi) * 128
                    lhsT = wt[0:128, c0:c0 + 128]
                    for s0, sn in _slices(lout):
                        nc.tensor.matmul(
                            ps[:, s0:s0 + sn], lhsT,
                            srcs[ti][0:128,
                                     2 * s0 + k:2 * s0 + k + 2 * sn:2],
                            start=ti == 0 and k == 0,
                            stop=ti == 1 and k == 3,
                            skip_group_check=True)
            act(ps[:, 0:lout], dst, i, mi)

        # ------------- DMAs -------------
        def dma_xr(lo, hi):
            for r in range(4):
                nc.sync.dma_start(out=xr4[32 * r:32 * r + 10, lo:hi],
                                  in_=xr_d.ap()[10 * r:10 * r + 10, lo:hi])

        dma_xr(0, 2048)
        w_early = WCOLS[(2, 'c1k128')]   # L0+L1 weights
        nc.sync.dma_start(out=wt[:, 0:w_early], in_=wp_d.ap()[:, 0:w_early])
        nc.sync.dma_start(out=bt[:, :], in_=bp_d.ap())
        dma_xr(2048, 8192)
        mid = WCOLS[(5, 'm')]
        nc.gpsimd.dma_start(out=wt[:, w_early:mid],
                            in_=wp_d.ap()[:, w_early:mid])
        nc.gpsimd.dma_start(out=wt[:, mid:WTOT], in_=wp_d.ap()[:, mid:WTOT])

        # ------------- emission -------------
        # L1/L2 pairs are processed with each layer's LAST pair first so
        # the halo columns (needed by the next layer's HIGH-half reads)
        # are available early, letting L2 units weave into the ACT-bound
        # L0/L1 phase to keep the PE dense (HAM clock-gate stays warm).
        junk_mms(10)
        np1 = len(_pairs(1))   # 8 pairs
        for g in (0, 7, 6, 5, 8, 15, 14, 13):
            u_l0(g)
        flush_acts()
        u_c1(1, np1 - 1, src_l1)
        u_c2(1, np1 - 1, src_l1)
        flush_acts()
        nc.vector.tensor_copy(t1[1][:, 1, 0:1], t1[1][:, 0, H[1]:H[1] + 1])
        nc.sync.dma_start(out=t2[1][64:128, 0:1],
                          in_=t2[1][0:64, H[1]:H[1] + 1])
        l0_rest = [1, 9, 2, 10, 3, 11, 4, 12]
        for P in range(np1 - 1):
            if 2 * P + 1 < len(l0_rest):
                u_l0(l0_rest[2 * P])
                u_l0(l0_rest[2 * P + 1])
            u_c1(1, P, src_l1)
            u_c2(1, P, src_l1)
            flush_acts()
            if P >= 3:
                u_c1(2, P - 3, src_t1(1))
            if P >= 5:
                u_c2(2, P - 5, src_t1(1))
        u_c2(2, 2, src_t1(1))
        u_c2(2, 3, src_t1(1))
        flush_acts()
        nc.vector.tensor_copy(t1[2][:, 1, 0:1], t1[2][:, 0, H[2]:H[2] + 1])
        nc.sync.dma_start(out=t2[2][64:128, 0:1],
                          in_=t2[2][0:64, H[2]:H[2] + 1])
        for p in range(2):
            u_c1(3, p, src_t1(2))
            u_c2(3, p, src_t1(2))
            flush_acts()
        nc.vector.tensor_copy(t1[3][:, 1, 0:1], t1[3][:, 0, H[3]:H[3] + 1])
        nc.sync.dma_start(out=t2[3][64:128, 0:1],
                          in_=t2[3][0:64, H[3]:H[3] + 1])
        u_l4(0)
        u_l4(1)
        flush_acts()
        u_tail(5, 0, a4, a5[0][:, 1:1001])
        u_tail(5, 1, a4, a5[1][:, 1:1001])
        flush_acts()
        u_tail(6, 0, a5, stage[:, 0:500])
        flush_acts()
        nc.sync.dma_start(out=y_d.ap()[0:128, :], in_=stage[:, 0:500])
        u_tail(6, 1, a5, stage[:, 500:1000])
        flush_acts()
        nc.sync.dma_start(out=y_d.ap()[128:256, :], in_=stage[:, 500:1000])

        if debug:
            F = mybir.dt.float16
            dbg_tiles = {"a0": a0, "t1_1": t1[1], "t2_1": t2[1],
                         "t1_2": t1[2], "t2_2": t2[2],
                         "t1_3": t1[3], "t2_3": t2[3],
                         "a4_0": a4[0], "a4_1": a4[1],
                         "a5_0": a5[0], "a5_1": a5[1]}
            shapes = {"a0": [128, LOUT[0] + 3],
                      "t1_1": [128, 2, H[1] + 4], "t2_1": [128, H[1] + 1],
                      "t1_2": [128, 2, H[2] + 4], "t2_2": [128, H[2] + 1],
                      "t1_3": [128, 2, H[3] + 4], "t2_3": [128, H[3] + 1],
                      "a4_0": [128, LOUT[4] + 4], "a4_1": [128, LOUT[4] + 4],
                      "a5_0": [128, LOUT[5] + 4], "a5_1": [128, LOUT[5] + 4]}
            for nm, tt in dbg_tiles.items():
                shp = shapes[nm]
                d = nc.dram_tensor("dbg_" + nm, shp, F,
                                   kind="ExternalOutput")
                dbg_d[nm] = d
                if len(shp) == 3:
                    nc.sync.dma_start(out=d.ap(), in_=tt[:, 0:2, :])
                else:
                    nc.sync.dma_start(out=d.ap(), in_=tt[:, :])

        pspool.release()
        for p in reversed(pools):
            p.release()

    nc.compile()
    _CACHE[("nc", debug)] = nc
    return nc


def kernel(x, w0, b0, w1, b1, w2, b2, w3, b3, w4, b4, w5, b5, w6, b6):
    import os
    from concourse.bass_utils import run_bass_kernel_spmd

    ws = [w0, w1, w2, w3, w4, w5, w6]
    bs = [b0, b1, b2, b3, b4, b5, b6]
    wpk, bpk = _pack_host(ws, bs)
    x = np.asarray(x, np.float32)
    in_maps = [{"xr": _prep_x(x[b]), "wp": wpk, "bp": bpk}
               for b in range(N_CORES)]
    debug = bool(os.environ.get("BITCONV_DEBUG"))
    nc = _build(debug=debug)
    trace = bool(os.environ.get("BITCONV_TRACE"))
    res = run_bass_kernel_spmd(nc, in_maps, core_ids=list(range(N_CORES)),
                               trace=trace)
    if trace or debug:
        if trace:
            print(f"HW exec time: {res.exec_time_ns} ns")
        _CACHE["last_results"] = res
    return np.stack([res.results[b]["y"] for b in range(N_CORES)], axis=0)


# revision 29
# speedup vs baseline: 1.0670x; 1.0670x over previous
"""Trainium2 Bass kernel for a 7-layer ternary-weight (BitNet) 1D conv
feature extractor with exact-erf GELU after each layer.

Contract: kernel(**inputs) takes the FULL inputs from setup_inputs()
(x: [8, 160000] f32, w0..w6 / b0..b6 conv params) and returns the full
output [8, 256, 500] f32.

v2 design (data-parallel, one batch element per core):
- L0 (Cin=1, K=10, stride 5) runs 4x row-tiled (tile_position=(32r,0),
  contraction 10): four 512-col output slices stream concurrently, so a
  2048-col output group costs ~512 PE cycles. Host preps a phase buffer
  xr4[10r+j, 512g+w] = xpad[5*(2048g+512r+w)+j].
- cout=192 layers (L1..L3) store channels 0..127 in a plain tile t1
  [128, 2, H+4] (dim1 = sequence half, one halo col) and channels
  128..191 in a packed tile t2 [128, H+1]: rows 0:64 = first half of
  the sequence, rows 64:128 = second half. t2 is produced with 2x
  column-tiled matmuls (both halves concurrently, M=64 each) and
  consumed with 2x row-tiled K=64 matmuls (both halves concurrently),
  keeping the PE array fully packed where v1 wasted half of it, and
  making every GELU a full 128-partition activation.
- Activations are [128, ~2048] (one 4-bank PSUM slot, 2-slot rotation)
  to amortize the ~293ns/instr ACT overhead.
- Emission interleaves L0 groups with L1 units (the ACT-bound phase) and
  streams L2..L6 as soon as their inputs are emitted.
"""

import numpy as np

# (in_ch, out_ch, kernel, stride, pad) -- fixed problem geometry
LAYERS = [(1, 128, 10, 5, 4), (128, 192, 3, 2, 1), (192, 192, 3, 2, 1),
          (192, 192, 3, 2, 1), (192, 256, 3, 2, 1), (256, 256, 4, 2, 1),
          (256, 256, 4, 2, 1)]
T_IN = 160000
LOUT = [32000, 16000, 8000, 4000, 2000, 1000, 500]
N_CORES = 8
NT = 512          # max matmul free dim / one fp32 PSUM bank
GW = 2048         # PSUM slot width (4 banks)
H = {i: LOUT[i] // 2 for i in (1, 2, 3)}  # packed-tile half lengths


def _slices(n, w=NT):
    return [(o, min(w, n - o)) for o in range(0, n, w)]


def _pairs(i):
    """c1-joint pair supertiles (t0, nst) over [0, H[i])."""
    return _slices(H[i], 1024)


# ---------------- host-side packing ----------------

def _wlayout():
    cols = {}
    tot = 0

    def add(key, n):
        nonlocal tot
        cols[key] = tot
        tot += n

    add((0,), 128)
    add((1, 'c1k128'), 3 * 128)
    add((1, 'c2k128'), 3 * 64)
    for i in (2, 3):
        add((i, 'c1k128'), 3 * 128)
        add((i, 'c1k64'), 3 * 128)
        add((i, 'c2k128'), 3 * 64)
        add((i, 'c2k64'), 3 * 64)
    add((4, 'c1k128'), 3 * 256)   # per tap: [mi=0 | mi=1]
    add((4, 'c1k64'), 3 * 256)
    for i in (5, 6):
        add((i, 'm'), 2 * 4 * 2 * 128)  # ti, k, mi major->minor
    return cols, tot


WCOLS, WTOT = _wlayout()
BCOLS = {}
_nb = 0
for _i in range(7):
    for _mi in range(2 if LAYERS[_i][1] > 128 else 1):
        BCOLS[(_i, _mi)] = _nb
        _nb += 2
NB = _nb


def _pack_host(ws, bs):
    wpk = np.zeros((128, WTOT), np.float16)
    bpk = np.zeros((128, NB), np.float32)
    signs = []
    for i in range(7):
        w = np.asarray(ws[i], np.float32)
        scale = max(float(np.mean(np.abs(w))), 1e-5)
        signs.append(np.clip(np.round(w / scale), -1.0, 1.0))
        b = np.asarray(bs[i], np.float32)
        cout = LAYERS[i][1]
        for mi in range(2 if cout > 128 else 1):
            m0 = 128 * mi
            msz = min(128, cout - m0)
            c = BCOLS[(i, mi)]
            bpk[0:msz, c] = b[m0:m0 + msz]
            bpk[0:msz, c + 1] = scale
            if msz == 64:  # packed chunk: duplicate on rows 64:128
                bpk[64:128, c] = b[m0:m0 + 64]
                bpk[64:128, c + 1] = scale
    f16 = np.float16
    s0 = signs[0][:, 0, :]  # [128, 10]
    for r in range(4):
        wpk[32 * r:32 * r + 10, WCOLS[(0,)]:WCOLS[(0,)] + 128] = \
            s0.T.astype(f16)
    for i in (1, 2, 3, 4):
        s = signs[i]
        cin, cout = LAYERS[i][0], LAYERS[i][1]
        nm1 = 2 if i == 4 else 1
        base = WCOLS[(i, 'c1k128')]
        for k in range(3):
            for mi in range(nm1):
                c0 = base + k * 128 * nm1 + mi * 128
                wpk[0:128, c0:c0 + 128] = \
                    s[128 * mi:128 * mi + 128, 0:128, k].T.astype(f16)
        if cin == 192:
            base = WCOLS[(i, 'c1k64')]
            for k in range(3):
                for mi in range(nm1):
                    c0 = base + k * 128 * nm1 + mi * 128
                    blk = s[128 * mi:128 * mi + 128, 128:192, k].T.astype(f16)
                    wpk[0:64, c0:c0 + 128] = blk
                    wpk[64:128, c0:c0 + 128] = blk
        if cout == 192:
            base = WCOLS[(i, 'c2k128')]
            for k in range(3):
                wpk[0:128, base + 64 * k:base + 64 * k + 64] = \
                    s[128:192, 0:128, k].T.astype(f16)
            if cin == 192:
                base = WCOLS[(i, 'c2k64')]
                for k in range(3):
                    blk = s[128:192, 128:192, k].T.astype(f16)
                    wpk[0:64, base + 64 * k:base + 64 * k + 64] = blk
                    wpk[64:128, base + 64 * k:base + 64 * k + 64] = blk
    for i in (5, 6):
        s = signs[i]
        base = WCOLS[(i, 'm')]
        n = 0
        for ti in range(2):
            for k in range(4):
                for mi in range(2):
                    wpk[0:128, base + n:base + n + 128] = \
                        s[128 * mi:128 * mi + 128,
                          128 * ti:128 * ti + 128, k].T.astype(f16)
                    n += 128
    return wpk, bpk


def _prep_x(xb):
    """xr4 [40, 8192]: xr4[10r+j, 512g+w] = xpad[5*(2048g+512r+w)+j]."""
    xpad = np.zeros(T_IN + 20, np.float16)
    xpad[4:4 + T_IN] = xb.astype(np.float16)
    xr = np.lib.stride_tricks.as_strided(
        xpad, shape=(10, LOUT[0]), strides=(2, 10))
    xr4 = np.zeros((40, 8192), np.float16)
    for g in range(16):
        for r in range(4):
            c0 = 2048 * g + 512 * r
            n = min(512, max(0, LOUT[0] - c0))
            if n:
                xr4[10 * r:10 * r + 10, 512 * g:512 * g + n] = \
                    xr[:, c0:c0 + n]
    return xr4


_CACHE = {}


def _build(debug=False):
    if ("nc", debug) in _CACHE:
        return _CACHE[("nc", debug)]
    from concourse import bacc
    import concourse.mybir as mybir
    import concourse.tile as tile

    F16 = mybir.dt.float16
    F32 = mybir.dt.float32
    GELU = mybir.ActivationFunctionType.Gelu

    nc = bacc.Bacc("TRN2")
    xr_d = nc.dram_tensor("xr", [40, 8192], F16, kind="ExternalInput")
    wp_d = nc.dram_tensor("wp", [128, WTOT], F16, kind="ExternalInput")
    bp_d = nc.dram_tensor("bp", [128, NB], F32, kind="ExternalInput")
    y_d = nc.dram_tensor("y", [256, 500], F32, kind="ExternalOutput")
    dbg_d = {}

    with tile.TileContext(nc) as tc:
        pools = []

        def mkpool(name, bufs=1, space="SBUF"):
            p = tc.alloc_tile_pool(name=name, bufs=bufs, space=space)
            pools.append(p)
            return p

        wpool = mkpool("wpool")
        wt = wpool.tile([128, WTOT], F16, name="wt")
        bt = wpool.tile([128, NB], F32, name="bt")
        scratch = wpool.tile([128, 512], F16, name="scratch")

        apool = mkpool("apool")
        xr4 = apool.tile([128, 8192], F16, name="xr4")
        a0 = apool.tile([128, LOUT[0] + 3], F16, name="a0")
        t1 = {i: apool.tile([128, 2, H[i] + 4], F16, name=f"t1_{i}")
              for i in (1, 2, 3)}
        t2 = {i: apool.tile([128, H[i] + 1], F16, name=f"t2_{i}")
              for i in (1, 2, 3)}
        a4 = [apool.tile([128, LOUT[4] + 4], F16, name=f"a4_{mi}")
              for mi in range(2)]
        a5 = [apool.tile([128, LOUT[5] + 4], F16, name=f"a5_{mi}")
              for mi in range(2)]
        stage = apool.tile([128, 1000], F32, name="stage")

        nc.vector.memset(scratch[:, :], 0.0)
        nc.vector.memset(a0[:, 0:1], 0.0)
        nc.vector.memset(a0[:, LOUT[0] + 1:LOUT[0] + 3], 0.0)
        for i in (1, 2, 3):
            nc.vector.memset(t1[i][:, 0:2, 0:1], 0.0)
            nc.vector.memset(t1[i][:, 0:2, H[i] + 1:H[i] + 3], 0.0)
            nc.vector.memset(t2[i][0:64, 0:1], 0.0)
        for t, L in [(a4[0], LOUT[4]), (a4[1], LOUT[4]),
                     (a5[0], LOUT[5]), (a5[1], LOUT[5])]:
            nc.vector.memset(t[:, 0:1], 0.0)
            nc.vector.memset(t[:, L + 1:L + 3], 0.0)

        pspool = tc.alloc_tile_pool(name="pspool", bufs=1, space="PSUM")
        psall = pspool.tile([128, 4, 1024], F32, name="psall")
        _cur = [0]

        def slots(n):
            """Claim n consecutive 1024-col PSUM slots (round-robin over
            4). Tile's region tracking orders reuse after the prior
            evacuation. Returns the slot index."""
            if _cur[0] % 4 + n > 4:
                _cur[0] += 4 - _cur[0] % 4
            base = _cur[0] % 4
            _cur[0] += n
            return base

        def junk_mms(n):
            for _ in range(n):
                nc.tensor.matmul(psall[:, 0, 0:512], scratch[0:128, 0:128],
                                 scratch[:, :], start=True, stop=True)

        def act(ps_ap, dst_ap, i, mi):
            c = BCOLS[(i, mi)]
            nc.scalar.activation(dst_ap, ps_ap, GELU,
                                 bias=bt[:, c:c + 1],
                                 scale=bt[:, c + 1:c + 2])

        def flush_acts():
            pass

        # ---- L0 unit: output cols [2048g, 2048g+2048) ----
        def u_l0(g):
            sl = slots(2)
            gn = min(GW, LOUT[0] - GW * g)
            for r in range(4):
                n = min(512, max(0, gn - 512 * r))
                if not n:
                    continue
                nc.tensor.matmul(
                    psall[:, sl + r // 2, (r % 2) * 512:(r % 2) * 512 + n],
                    wt[32 * r:32 * r + 10, WCOLS[(0,)]:WCOLS[(0,)] + 128],
                    xr4[32 * r:32 * r + 10, 512 * g:512 * g + n],
                    start=True, stop=True, tile_position=(32 * r, 0))
            if gn == GW:
                act(psall[:, sl:sl + 2, :],
                    a0[:, 1 + GW * g:1 + GW * g + gn], 0, 0)
            else:
                n0 = min(gn, 1024)
                act(psall[:, sl, 0:n0],
                    a0[:, 1 + GW * g:1 + GW * g + n0], 0, 0)
                if gn > 1024:
                    act(psall[:, sl + 1, 0:gn - 1024],
                        a0[:, 1 + GW * g + 1024:1 + GW * g + gn], 0, 0)

        # ---- source AP builders (h = sequence half, local out col t0) --
        def src_l1(h, k, t0, n):
            c = 16000 * h + 2 * t0 + k
            return a0[0:128, c:c + 2 * n:2]

        def src_t1(i):
            def f(h, k, t0, n):
                c = 2 * t0 + k
                return t1[i][0:128, h, c:c + 2 * n:2]
            return f

        # ---- c1-joint unit for layer i in (1,2,3): both halves of the
        # 128-channel chunk for pair p ----
        def u_c1(i, p, src):
            t0, nst = _pairs(i)[p]
            cin = LAYERS[i][0]
            sl = slots(2)
            ps = psall[:, sl:sl + 2, :]
            cols = WCOLS[(i, 'c1k128')]
            for k in range(3):
                lhsT = wt[0:128, cols + 128 * k:cols + 128 * k + 128]
                for h in range(2):
                    for s0, sn in _slices(nst):
                        nc.tensor.matmul(
                            ps[:, h, s0:s0 + sn], lhsT,
                            src(h, k, t0 + s0, sn),
                            start=k == 0, stop=cin == 128 and k == 2,
                            skip_group_check=True)
            if cin == 192:
                cols64 = WCOLS[(i, 'c1k64')]
                t2p = t2[i - 1]
                for k in range(3):
                    c0 = cols64 + 128 * k
                    for s0, sn in _slices(nst):
                        for h in range(2):
                            nc.tensor.matmul(
                                ps[:, h, s0:s0 + sn],
                                wt[64 * h:64 * h + 64, c0:c0 + 128],
                                t2p[64 * h:64 * h + 64,
                                    2 * (t0 + s0) + k:
                                    2 * (t0 + s0) + k + 2 * sn - 1:2],
                                start=False, stop=k == 2,
                                skip_group_check=True,
                                tile_position=(64 * h, 0))
            act(ps[:, 0:2, 0:nst], t1[i][:, 0:2, 1 + t0:1 + t0 + nst], i, 0)

        # ---- packed c2 unit for layer i in (1,2,3): one pair p ----
        def u_c2(i, p, src):
            t0, nst = _pairs(i)[p]
            cin = LAYERS[i][0]
            sl = slots(1)
            ps = psall[:, sl, :]
            ck = WCOLS[(i, 'c2k128')]
            for k in range(3):
                lhsT = wt[0:128, ck + 64 * k:ck + 64 * k + 64]
                for s0, sn in _slices(nst):
                    for h in range(2):
                        nc.tensor.matmul(
                            ps[64 * h:64 * h + 64, s0:s0 + sn],
                            lhsT, src(h, k, t0 + s0, sn),
                            start=k == 0,
                            stop=cin == 128 and k == 2 and h == 1,
                            skip_group_check=True,
                            tile_position=(0, 64 * h))
            if cin == 192:
                ck64 = WCOLS[(i, 'c2k64')]
                t2p = t2[i - 1]
                for k in range(3):
                    for s0, sn in _slices(nst):
                        for h in range(2):
                            nc.tensor.matmul(
                                ps[64 * h:64 * h + 64, s0:s0 + sn],
                                wt[64 * h:64 * h + 64,
                                   ck64 + 64 * k:ck64 + 64 * k + 64],
                                t2p[64 * h:64 * h + 64,
                                    2 * (t0 + s0) + k:
                                    2 * (t0 + s0) + k + 2 * sn - 1:2],
                                start=False,
                                stop=k == 2 and h == 1,
                                skip_group_check=True,
                                tile_position=(64 * h, 64 * h))
            act(ps[:, 0:nst], t2[i][:, 1 + t0:1 + t0 + nst], i, 1)

        # ---- L4 unit (chunk mi): 2000 out cols, halves of 1000 ----
        def u_l4(mi):
            sl = slots(2)
            ps = psall[:, sl:sl + 2, :]
            cols = WCOLS[(4, 'c1k128')]
            cols64 = WCOLS[(4, 'c1k64')]
            for k in range(3):
                lhsT = wt[0:128, cols + k * 256 + 128 * mi:
                          cols + k * 256 + 128 * mi + 128]
                for h in range(2):
                    for s0, sn in _slices(1000):
                        nc.tensor.matmul(
                            ps[:, h, s0:s0 + sn], lhsT,
                            src_t1(3)(h, k, s0, sn),
                            start=k == 0, stop=False,
                            skip_group_check=True)
            for k in range(3):
                c0 = cols64 + k * 256 + 128 * mi
                for s0, sn in _slices(1000):
                    for h in range(2):
                        nc.tensor.matmul(
                            ps[:, h, s0:s0 + sn],
                            wt[64 * h:64 * h + 64, c0:c0 + 128],
                            t2[3][64 * h:64 * h + 64,
                                  2 * s0 + k:2 * s0 + k + 2 * sn - 1:2],
                            start=False, stop=k == 2,
                            skip_group_check=True,
                            tile_position=(64 * h, 0))
            act(ps[:, 0, 0:1000], a4[mi][:, 1:1001], 4, mi)
            act(ps[:, 1, 0:1000], a4[mi][:, 1001:2001], 4, mi)

        # ---- L5/L6 plain units ----
        def u_tail(i, mi, srcs, dst):
            lout = LOUT[i]
            sl = slots(1)
            ps = psall[:, sl, :]
            base = WCOLS[(i, 'm')]
            for ti in range(2):
                for k in range(4):
                    c0 = base + (ti * 8 + k * 2 + mi) * 128
                    lhsT = wt[0:128, c0:c0 + 128]
                    for s0, sn in _slices(lout):
                        nc.tensor.matmul(
                            ps[:, s0:s0 + sn], lhsT,
                            srcs[ti][0:128,
                                     2 * s0 + k:2 * s0 + k + 2 * sn:2],
                            start=ti == 0 and k == 0,
                            stop=ti == 1 and k == 3,
                            skip_group_check=True)
            act(ps[:, 0:lout], dst, i, mi)

        # ------------- DMAs -------------
        def dma_xr(lo, hi):
            for r in range(4):
                nc.sync.dma_start(out=xr4[32 * r:32 * r + 10, lo:hi],
                                  in_=xr_d.ap()[10 * r:10 * r + 10, lo:hi])

        dma_xr(0, 2048)
        w_early = WCOLS[(2, 'c1k128')]   # L0+L1 weights
        nc.sync.dma_start(out=wt[:, 0:w_early], in_=wp_d.ap()[:, 0:w_early])
        nc.sync.dma_start(out=bt[:, :], in_=bp_d.ap())
        dma_xr(2048, 8192)
        mid = WCOLS[(5, 'm')]
        nc.gpsimd.dma_start(out=wt[:, w_early:mid],
                            in_=wp_d.ap()[:, w_early:mid])
        nc.gpsimd.dma_start(out=wt[:, mid:WTOT], in_=wp_d.ap()[:, mid:WTOT])

        # ------------- emission -------------
        # L1/L2 pairs are processed with each layer's LAST pair first so
        # the halo columns (needed by the next layer's HIGH-half reads)
        # are available early, letting L2 units weave into the ACT-bound
        # L0/L1 phase to keep the PE dense (HAM clock-gate stays warm).
        junk_mms(10)
        np1 = len(_pairs(1))   # 8 pairs
        for g in (0, 7, 6, 5, 8, 15, 14, 13):
            u_l0(g)
        flush_acts()
        u_c1(1, np1 - 1, src_l1)
        u_c2(1, np1 - 1, src_l1)
        flush_acts()
        nc.vector.tensor_copy(t1[1][:, 1, 0:1], t1[1][:, 0, H[1]:H[1] + 1])
        nc.sync.dma_start(out=t2[1][64:128, 0:1],
                          in_=t2[1][0:64, H[1]:H[1] + 1])
        l0_rest = [1, 9, 2, 10, 3, 11, 4, 12]
        for P in range(np1 - 1):
            if 2 * P + 1 < len(l0_rest):
                u_l0(l0_rest[2 * P])
                u_l0(l0_rest[2 * P + 1])
            u_c1(1, P, src_l1)
            u_c2(1, P, src_l1)
            flush_acts()
            if P >= 3:
                u_c1(2, P - 3, src_t1(1))
            if P >= 5:
                u_c2(2, P - 5, src_t1(1))
        u_c2(2, 2, src_t1(1))
        u_c2(2, 3, src_t1(1))
        flush_acts()
        nc.vector.tensor_copy(t1[2][:, 1, 0:1], t1[2][:, 0, H[2]:H[2] + 1])
        nc.sync.dma_start(out=t2[2][64:128, 0:1],
                          in_=t2[2][0:64, H[2]:H[2] + 1])
        for p in range(2):
            u_c1(3, p, src_t1(2))
            u_c2(3, p, src_t1(2))
            flush_acts()
        nc.vector.tensor_copy(t1[3][:, 1, 0:1], t1[3][:, 0, H[3]:H[3] + 1])
        nc.sync.dma_start(out=t2[3][64:128, 0:1],
                          in_=t2[3][0:64, H[3]:H[3] + 1])
        u_l4(0)
        u_l4(1)
        flush_acts()
        u_tail(5, 0, a4, a5[0][:, 1:1001])
        u_tail(5, 1, a4, a5[1][:, 1:1001])
        flush_acts()
        u_tail(6, 0, a5, stage[:, 0:500])
        flush_acts()
        nc.sync.dma_start(out=y_d.ap()[0:128, :], in_=stage[:, 0:500])
        u_tail(6, 1, a5, stage[:, 500:1000])
        flush_acts()
        nc.sync.dma_start(out=y_d.ap()[128:256, :], in_=stage[:, 500:1000])

        if debug:
            F = mybir.dt.float16
            dbg_tiles = {"a0": a0, "t1_1": t1[1], "t2_1": t2[1],
                         "t1_2": t1[2], "t2_2": t2[2],
                         "t1_3": t1[3], "t2_3": t2[3],
                         "a4_0": a4[0], "a4_1": a4[1],
                         "a5_0": a5[0], "a5_1": a5[1]}
            shapes = {"a0": [128, LOUT[0] + 3],
                      "t1_1": [128, 2, H[1] + 4], "t2_1": [128, H[1] + 1],
                      "t1_2": [128, 2, H[2] + 4], "t2_2": [128, H[2] + 1],
                      "t1_3": [128, 2, H[3] + 4], "t2_3": [128, H[3] + 1],
                      "a4_0": [128, LOUT[4] + 4], "a4_1": [128, LOUT[4] + 4],
                      "a5_0": [128, LOUT[5] + 4], "a5_1": [128, LOUT[5] + 4]}
            for nm, tt in dbg_tiles.items():
                shp = shapes[nm]
                d = nc.dram_tensor("dbg_" + nm, shp, F,
                                   kind="ExternalOutput")
                dbg_d[nm] = d
                if len(shp) == 3:
                    nc.sync.dma_start(out=d.ap(), in_=tt[:, 0:2, :])
                else:
                    nc.sync.dma_start(out=d.ap(), in_=tt[:, :])

        pspool.release()
        for p in reversed(pools):
            p.release()

    nc.compile()
    _CACHE[("nc", debug)] = nc
    return nc


def kernel(x, w0, b0, w1, b1, w2, b2, w3, b3, w4, b4, w5, b5, w6, b6):
    import os
    from concourse.bass_utils import run_bass_kernel_spmd

    ws = [w0, w1, w2, w3, w4, w5, w6]
    bs = [b0, b1, b2, b3, b4, b5, b6]
    wpk, bpk = _pack_host(ws, bs)
    x = np.asarray(x, np.float32)
    in_maps = [{"xr": _prep_x(x[b]), "wp": wpk, "bp": bpk}
               for b in range(N_CORES)]
    debug = bool(os.environ.get("BITCONV_DEBUG"))
    nc = _build(debug=debug)
    trace = bool(os.environ.get("BITCONV_TRACE"))
    res = run_bass_kernel_spmd(nc, in_maps, core_ids=list(range(N_CORES)),
                               trace=trace)
    if trace or debug:
        if trace:
            print(f"HW exec time: {res.exec_time_ns} ns")
        _CACHE["last_results"] = res
    return np.stack([res.results[b]["y"] for b in range(N_CORES)], axis=0)


# revision 30
# speedup vs baseline: 1.0718x; 1.0045x over previous
"""Trainium2 Bass kernel for a 7-layer ternary-weight (BitNet) 1D conv
feature extractor with exact-erf GELU after each layer.

Contract: kernel(**inputs) takes the FULL inputs from setup_inputs()
(x: [8, 160000] f32, w0..w6 / b0..b6 conv params) and returns the full
output [8, 256, 500] f32.

v2 design (data-parallel, one batch element per core):
- L0 (Cin=1, K=10, stride 5) runs 4x row-tiled (tile_position=(32r,0),
  contraction 10): four 512-col output slices stream concurrently, so a
  2048-col output group costs ~512 PE cycles. Host preps a phase buffer
  xr4[10r+j, 512g+w] = xpad[5*(2048g+512r+w)+j].
- cout=192 layers (L1..L3) store channels 0..127 in a plain tile t1
  [128, 2, H+4] (dim1 = sequence half, one halo col) and channels
  128..191 in a packed tile t2 [128, H+1]: rows 0:64 = first half of
  the sequence, rows 64:128 = second half. t2 is produced with 2x
  column-tiled matmuls (both halves concurrently, M=64 each) and
  consumed with 2x row-tiled K=64 matmuls (both halves concurrently),
  keeping the PE array fully packed where v1 wasted half of it, and
  making every GELU a full 128-partition activation.
- Activations are [128, ~2048] reading a manually rotated 4x1024-col
  PSUM slot ring, amortizing the ~293ns/instr ACT overhead.
- Emission processes each layer's LAST pair first so the halo columns
  (needed by the next layer's HIGH-half reads) exist early; L2 units
  weave into the L0/L1 stream to keep the PE dense (HAM clock at 8/8).
  Known next step (not implemented): split each batch element into two
  independent half-sequence networks per core (host-side halo margins)
  and interleave their units, doubling the effective PSUM pipeline
  depth that currently gates both PE and ACT occupancy.
"""

import numpy as np

# (in_ch, out_ch, kernel, stride, pad) -- fixed problem geometry
LAYERS = [(1, 128, 10, 5, 4), (128, 192, 3, 2, 1), (192, 192, 3, 2, 1),
          (192, 192, 3, 2, 1), (192, 256, 3, 2, 1), (256, 256, 4, 2, 1),
          (256, 256, 4, 2, 1)]
T_IN = 160000
LOUT = [32000, 16000, 8000, 4000, 2000, 1000, 500]
N_CORES = 8
NT = 512          # max matmul free dim / one fp32 PSUM bank
GW = 2048         # PSUM slot width (4 banks)
H = {i: LOUT[i] // 2 for i in (1, 2, 3)}  # packed-tile half lengths


def _slices(n, w=NT):
    return [(o, min(w, n - o)) for o in range(0, n, w)]


def _pairs(i):
    """c1-joint pair supertiles (t0, nst) over [0, H[i])."""
    return _slices(H[i], 1024)


# ---------------- host-side packing ----------------

def _wlayout():
    cols = {}
    tot = 0

    def add(key, n):
        nonlocal tot
        cols[key] = tot
        tot += n

    add((0,), 128)
    add((1, 'c1k128'), 3 * 128)
    add((1, 'c2k128'), 3 * 64)
    for i in (2, 3):
        add((i, 'c1k128'), 3 * 128)
        add((i, 'c1k64'), 3 * 128)
        add((i, 'c2k128'), 3 * 64)
        add((i, 'c2k64'), 3 * 64)
    add((4, 'c1k128'), 3 * 256)   # per tap: [mi=0 | mi=1]
    add((4, 'c1k64'), 3 * 256)
    for i in (5, 6):
        add((i, 'm'), 2 * 4 * 2 * 128)  # ti, k, mi major->minor
    return cols, tot


WCOLS, WTOT = _wlayout()
BCOLS = {}
_nb = 0
for _i in range(7):
    for _mi in range(2 if LAYERS[_i][1] > 128 else 1):
        BCOLS[(_i, _mi)] = _nb
        _nb += 2
NB = _nb


def _pack_host(ws, bs):
    wpk = np.zeros((128, WTOT), np.float16)
    bpk = np.zeros((128, NB), np.float32)
    signs = []
    for i in range(7):
        w = np.asarray(ws[i], np.float32)
        scale = max(float(np.mean(np.abs(w))), 1e-5)
        signs.append(np.clip(np.round(w / scale), -1.0, 1.0))
        b = np.asarray(bs[i], np.float32)
        cout = LAYERS[i][1]
        for mi in range(2 if cout > 128 else 1):
            m0 = 128 * mi
            msz = min(128, cout - m0)
            c = BCOLS[(i, mi)]
            bpk[0:msz, c] = b[m0:m0 + msz]
            bpk[0:msz, c + 1] = scale
            if msz == 64:  # packed chunk: duplicate on rows 64:128
                bpk[64:128, c] = b[m0:m0 + 64]
                bpk[64:128, c + 1] = scale
    f16 = np.float16
    s0 = signs[0][:, 0, :]  # [128, 10]
    for r in range(4):
        wpk[32 * r:32 * r + 10, WCOLS[(0,)]:WCOLS[(0,)] + 128] = \
            s0.T.astype(f16)
    for i in (1, 2, 3, 4):
        s = signs[i]
        cin, cout = LAYERS[i][0], LAYERS[i][1]
        nm1 = 2 if i == 4 else 1
        base = WCOLS[(i, 'c1k128')]
        for k in range(3):
            for mi in range(nm1):
                c0 = base + k * 128 * nm1 + mi * 128
                wpk[0:128, c0:c0 + 128] = \
                    s[128 * mi:128 * mi + 128, 0:128, k].T.astype(f16)
        if cin == 192:
            base = WCOLS[(i, 'c1k64')]
            for k in range(3):
                for mi in range(nm1):
                    c0 = base + k * 128 * nm1 + mi * 128
                    blk = s[128 * mi:128 * mi + 128, 128:192, k].T.astype(f16)
                    wpk[0:64, c0:c0 + 128] = blk
                    wpk[64:128, c0:c0 + 128] = blk
        if cout == 192:
            base = WCOLS[(i, 'c2k128')]
            for k in range(3):
                wpk[0:128, base + 64 * k:base + 64 * k + 64] = \
                    s[128:192, 0:128, k].T.astype(f16)
            if cin == 192:
                base = WCOLS[(i, 'c2k64')]
                for k in range(3):
                    blk = s[128:192, 128:192, k].T.astype(f16)
                    wpk[0:64, base + 64 * k:base + 64 * k + 64] = blk
                    wpk[64:128, base + 64 * k:base + 64 * k + 64] = blk
    for i in (5, 6):
        s = signs[i]
        base = WCOLS[(i, 'm')]
        n = 0
        for ti in range(2):
            for k in range(4):
                for mi in range(2):
                    wpk[0:128, base + n:base + n + 128] = \
                        s[128 * mi:128 * mi + 128,
                          128 * ti:128 * ti + 128, k].T.astype(f16)
                    n += 128
    return wpk, bpk


def _prep_x(xb):
    """xr4 [40, 8192]: xr4[10r+j, 512g+w] = xpad[5*(2048g+512r+w)+j]."""
    xpad = np.zeros(T_IN + 20, np.float16)
    xpad[4:4 + T_IN] = xb.astype(np.float16)
    xr = np.lib.stride_tricks.as_strided(
        xpad, shape=(10, LOUT[0]), strides=(2, 10))
    xr4 = np.zeros((40, 8192), np.float16)
    for g in range(16):
        for r in range(4):
            c0 = 2048 * g + 512 * r
            n = min(512, max(0, LOUT[0] - c0))
            if n:
                xr4[10 * r:10 * r + 10, 512 * g:512 * g + n] = \
                    xr[:, c0:c0 + n]
    return xr4


_CACHE = {}


def _build(debug=False):
    if ("nc", debug) in _CACHE:
        return _CACHE[("nc", debug)]
    from concourse import bacc
    import concourse.mybir as mybir
    import concourse.tile as tile

    F16 = mybir.dt.float16
    F32 = mybir.dt.float32
    GELU = mybir.ActivationFunctionType.Gelu

    nc = bacc.Bacc("TRN2")
    xr_d = nc.dram_tensor("xr", [40, 8192], F16, kind="ExternalInput")
    wp_d = nc.dram_tensor("wp", [128, WTOT], F16, kind="ExternalInput")
    bp_d = nc.dram_tensor("bp", [128, NB], F32, kind="ExternalInput")
    y_d = nc.dram_tensor("y", [256, 500], F32, kind="ExternalOutput")
    dbg_d = {}

    with tile.TileContext(nc) as tc:
        pools = []

        def mkpool(name, bufs=1, space="SBUF"):
            p = tc.alloc_tile_pool(name=name, bufs=bufs, space=space)
            pools.append(p)
            return p

        wpool = mkpool("wpool")
        wt = wpool.tile([128, WTOT], F16, name="wt")
        bt = wpool.tile([128, NB], F32, name="bt")
        scratch = wpool.tile([128, 512], F16, name="scratch")

        apool = mkpool("apool")
        xr4 = apool.tile([128, 8192], F16, name="xr4")
        a0 = apool.tile([128, LOUT[0] + 3], F16, name="a0")
        t1 = {i: apool.tile([128, 2, H[i] + 4], F16, name=f"t1_{i}")
              for i in (1, 2, 3)}
        t2 = {i: apool.tile([128, H[i] + 1], F16, name=f"t2_{i}")
              for i in (1, 2, 3)}
        a4 = [apool.tile([128, LOUT[4] + 4], F16, name=f"a4_{mi}")
              for mi in range(2)]
        a5 = [apool.tile([128, LOUT[5] + 4], F16, name=f"a5_{mi}")
              for mi in range(2)]
        stage = apool.tile([128, 1000], F32, name="stage")

        nc.vector.memset(scratch[:, :], 0.0)
        nc.vector.memset(a0[:, 0:1], 0.0)
        nc.vector.memset(a0[:, LOUT[0] + 1:LOUT[0] + 3], 0.0)
        for i in (1, 2, 3):
            nc.vector.memset(t1[i][:, 0:2, 0:1], 0.0)
            nc.vector.memset(t1[i][:, 0:2, H[i] + 1:H[i] + 3], 0.0)
            nc.vector.memset(t2[i][0:64, 0:1], 0.0)
        for t, L in [(a4[0], LOUT[4]), (a4[1], LOUT[4]),
                     (a5[0], LOUT[5]), (a5[1], LOUT[5])]:
            nc.vector.memset(t[:, 0:1], 0.0)
            nc.vector.memset(t[:, L + 1:L + 3], 0.0)

        pspool = tc.alloc_tile_pool(name="pspool", bufs=1, space="PSUM")
        psall = pspool.tile([128, 4, 1024], F32, name="psall")
        _cur = [0]

        def slots(n):
            """Claim n consecutive 1024-col PSUM slots (round-robin over
            4). Tile's region tracking orders reuse after the prior
            evacuation. Returns the slot index."""
            if _cur[0] % 4 + n > 4:
                _cur[0] += 4 - _cur[0] % 4
            base = _cur[0] % 4
            _cur[0] += n
            return base

        def junk_mms(n):
            for _ in range(n):
                nc.tensor.matmul(psall[:, 0, 0:512], scratch[0:128, 0:128],
                                 scratch[:, :], start=True, stop=True)

        def act(ps_ap, dst_ap, i, mi):
            c = BCOLS[(i, mi)]
            nc.scalar.activation(dst_ap, ps_ap, GELU,
                                 bias=bt[:, c:c + 1],
                                 scale=bt[:, c + 1:c + 2])

        def flush_acts():
            pass

        # ---- L0 unit: output cols [2048g, 2048g+2048) ----
        def u_l0(g):
            sl = slots(2)
            gn = min(GW, LOUT[0] - GW * g)
            for r in range(4):
                n = min(512, max(0, gn - 512 * r))
                if not n:
                    continue
                nc.tensor.matmul(
                    psall[:, sl + r // 2, (r % 2) * 512:(r % 2) * 512 + n],
                    wt[32 * r:32 * r + 10, WCOLS[(0,)]:WCOLS[(0,)] + 128],
                    xr4[32 * r:32 * r + 10, 512 * g:512 * g + n],
                    start=True, stop=True, tile_position=(32 * r, 0))
            if gn == GW:
                act(psall[:, sl:sl + 2, :],
                    a0[:, 1 + GW * g:1 + GW * g + gn], 0, 0)
            else:
                n0 = min(gn, 1024)
                act(psall[:, sl, 0:n0],
                    a0[:, 1 + GW * g:1 + GW * g + n0], 0, 0)
                if gn > 1024:
                    act(psall[:, sl + 1, 0:gn - 1024],
                        a0[:, 1 + GW * g + 1024:1 + GW * g + gn], 0, 0)

        # ---- source AP builders (h = sequence half, local out col t0) --
        def src_l1(h, k, t0, n):
            c = 16000 * h + 2 * t0 + k
            return a0[0:128, c:c + 2 * n:2]

        def src_t1(i):
            def f(h, k, t0, n):
                c = 2 * t0 + k
                return t1[i][0:128, h, c:c + 2 * n:2]
            return f

        # ---- c1-joint unit for layer i in (1,2,3): both halves of the
        # 128-channel chunk for pair p ----
        def u_c1(i, p, src):
            t0, nst = _pairs(i)[p]
            cin = LAYERS[i][0]
            sl = slots(2)
            ps = psall[:, sl:sl + 2, :]
            cols = WCOLS[(i, 'c1k128')]
            for k in range(3):
                lhsT = wt[0:128, cols + 128 * k:cols + 128 * k + 128]
                for h in range(2):
                    for s0, sn in _slices(nst):
                        nc.tensor.matmul(
                            ps[:, h, s0:s0 + sn], lhsT,
                            src(h, k, t0 + s0, sn),
                            start=k == 0, stop=cin == 128 and k == 2,
                            skip_group_check=True)
            if cin == 192:
                cols64 = WCOLS[(i, 'c1k64')]
                t2p = t2[i - 1]
                for k in range(3):
                    c0 = cols64 + 128 * k
                    for s0, sn in _slices(nst):
                        for h in range(2):
                            nc.tensor.matmul(
                                ps[:, h, s0:s0 + sn],
                                wt[64 * h:64 * h + 64, c0:c0 + 128],
                                t2p[64 * h:64 * h + 64,
                                    2 * (t0 + s0) + k:
                                    2 * (t0 + s0) + k + 2 * sn - 1:2],
                                start=False, stop=k == 2,
                                skip_group_check=True,
                                tile_position=(64 * h, 0))
            act(ps[:, 0:2, 0:nst], t1[i][:, 0:2, 1 + t0:1 + t0 + nst], i, 0)

        # ---- packed c2 unit for layer i in (1,2,3): one pair p ----
        def u_c2(i, p, src):
            t0, nst = _pairs(i)[p]
            cin = LAYERS[i][0]
            sl = slots(1)
            ps = psall[:, sl, :]
            ck = WCOLS[(i, 'c2k128')]
            for k in range(3):
                lhsT = wt[0:128, ck + 64 * k:ck + 64 * k + 64]
                for s0, sn in _slices(nst):
                    for h in range(2):
                        nc.tensor.matmul(
                            ps[64 * h:64 * h + 64, s0:s0 + sn],
                            lhsT, src(h, k, t0 + s0, sn),
                            start=k == 0,
                            stop=cin == 128 and k == 2 and h == 1,
                            skip_group_check=True,
                            tile_position=(0, 64 * h))
            if cin == 192:
                ck64 = WCOLS[(i, 'c2k64')]
                t2p = t2[i - 1]
                for k in range(3):
                    for s0, sn in _slices(nst):
                        for h in range(2):
                            nc.tensor.matmul(
                                ps[64 * h:64 * h + 64, s0:s0 + sn],
                                wt[64 * h:64 * h + 64,
                                   ck64 + 64 * k:ck64 + 64 * k + 64],
                                t2p[64 * h:64 * h + 64,
                                    2 * (t0 + s0) + k:
                                    2 * (t0 + s0) + k + 2 * sn - 1:2],
                                start=False,
                                stop=k == 2 and h == 1,
                                skip_group_check=True,
                                tile_position=(64 * h, 64 * h))
            act(ps[:, 0:nst], t2[i][:, 1 + t0:1 + t0 + nst], i, 1)

        # ---- L4 unit (chunk mi): 2000 out cols, halves of 1000 ----
        def u_l4(mi):
            sl = slots(2)
            ps = psall[:, sl:sl + 2, :]
            cols = WCOLS[(4, 'c1k128')]
            cols64 = WCOLS[(4, 'c1k64')]
            for k in range(3):
                lhsT = wt[0:128, cols + k * 256 + 128 * mi:
                          cols + k * 256 + 128 * mi + 128]
                for h in range(2):
                    for s0, sn in _slices(1000):
                        nc.tensor.matmul(
                            ps[:, h, s0:s0 + sn], lhsT,
                            src_t1(3)(h, k, s0, sn),
                            start=k == 0, stop=False,
                            skip_group_check=True)
            for k in range(3):
                c0 = cols64 + k * 256 + 128 * mi
                for s0, sn in _slices(1000):
                    for h in range(2):
                        nc.tensor.matmul(
                            ps[:, h, s0:s0 + sn],
                            wt[64 * h:64 * h + 64, c0:c0 + 128],
                            t2[3][64 * h:64 * h + 64,
                                  2 * s0 + k:2 * s0 + k + 2 * sn - 1:2],
                            start=False, stop=k == 2,
                            skip_group_check=True,
                            tile_position=(64 * h, 0))
            act(ps[:, 0, 0:1000], a4[mi][:, 1:1001], 4, mi)
            act(ps[:, 1, 0:1000], a4[mi][:, 1001:2001], 4, mi)

        # ---- L5/L6 plain units ----
        def u_tail(i, mi, srcs, dst):
            lout = LOUT[i]
            sl = slots(1)
            ps = psall[:, sl, :]
            base = WCOLS[(i, 'm')]
            for ti in range(2):
                for k in range(4):
                    c0 = base + (ti * 8 + k * 2 + mi) * 128
                    lhsT = wt[0:128, c0:c0 + 128]
                    for s0, sn in _slices(lout):
                        nc.tensor.matmul(
                            ps[:, s0:s0 + sn], lhsT,
                            srcs[ti][0:128,
                                     2 * s0 + k:2 * s0 + k + 2 * sn:2],
                            start=ti == 0 and k == 0,
                            stop=ti == 1 and k == 3,
                            skip_group_check=True)
            act(ps[:, 0:lout], dst, i, mi)

        # ------------- DMAs -------------
        def dma_xr(lo, hi):
            for r in range(4):
                nc.sync.dma_start(out=xr4[32 * r:32 * r + 10, lo:hi],
                                  in_=xr_d.ap()[10 * r:10 * r + 10, lo:hi])

        dma_xr(0, 2048)
        w_early = WCOLS[(2, 'c1k128')]   # L0+L1 weights
        nc.sync.dma_start(out=wt[:, 0:w_early], in_=wp_d.ap()[:, 0:w_early])
        nc.sync.dma_start(out=bt[:, :], in_=bp_d.ap())
        dma_xr(2048, 8192)
        mid = WCOLS[(5, 'm')]
        nc.gpsimd.dma_start(out=wt[:, w_early:mid],
                            in_=wp_d.ap()[:, w_early:mid])
        nc.gpsimd.dma_start(out=wt[:, mid:WTOT], in_=wp_d.ap()[:, mid:WTOT])

        # ------------- emission -------------
        # L1/L2 pairs are processed with each layer's LAST pair first so
        # the halo columns (needed by the next layer's HIGH-half reads)
        # are available early, letting L2 units weave into the ACT-bound
        # L0/L1 phase to keep the PE dense (HAM clock-gate stays warm).
        junk_mms(10)
        np1 = len(_pairs(1))   # 8 pairs
        for g in (0, 7, 6, 5, 8, 15, 14, 13):
            u_l0(g)
        flush_acts()
        u_c1(1, np1 - 1, src_l1)
        u_c2(1, np1 - 1, src_l1)
        flush_acts()
        nc.vector.tensor_copy(t1[1][:, 1, 0:1], t1[1][:, 0, H[1]:H[1] + 1])
        nc.sync.dma_start(out=t2[1][64:128, 0:1],
                          in_=t2[1][0:64, H[1]:H[1] + 1])
        l0_rest = [1, 9, 2, 10, 3, 11, 4, 12]
        for P in range(np1 - 1):
            if 2 * P + 1 < len(l0_rest):
                u_l0(l0_rest[2 * P])
                u_l0(l0_rest[2 * P + 1])
            u_c1(1, P, src_l1)
            u_c2(1, P, src_l1)
            flush_acts()
            if P >= 3:
                u_c1(2, P - 3, src_t1(1))
            if P >= 5:
                u_c2(2, P - 5, src_t1(1))
        u_c2(2, 2, src_t1(1))
        u_c2(2, 3, src_t1(1))
        flush_acts()
        nc.vector.tensor_copy(t1[2][:, 1, 0:1], t1[2][:, 0, H[2]:H[2] + 1])
        nc.sync.dma_start(out=t2[2][64:128, 0:1],
                          in_=t2[2][0:64, H[2]:H[2] + 1])
        for p in range(2):
            u_c1(3, p, src_t1(2))
            u_c2(3, p, src_t1(2))
            flush_acts()
        nc.vector.tensor_copy(t1[3][:, 1, 0:1], t1[3][:, 0, H[3]:H[3] + 1])
        nc.sync.dma_start(out=t2[3][64:128, 0:1],
                          in_=t2[3][0:64, H[3]:H[3] + 1])
        u_l4(0)
        u_l4(1)
        flush_acts()
        u_tail(5, 0, a4, a5[0][:, 1:1001])
        u_tail(5, 1, a4, a5[1][:, 1:1001])
        flush_acts()
        u_tail(6, 0, a5, stage[:, 0:500])
        flush_acts()
        nc.sync.dma_start(out=y_d.ap()[0:128, :], in_=stage[:, 0:500])
        u_tail(6, 1, a5, stage[:, 500:1000])
        flush_acts()
        nc.sync.dma_start(out=y_d.ap()[128:256, :], in_=stage[:, 500:1000])

        if debug:
            F = mybir.dt.float16
            dbg_tiles = {"a0": a0, "t1_1": t1[1], "t2_1": t2[1],
                         "t1_2": t1[2], "t2_2": t2[2],
                         "t1_3": t1[3], "t2_3": t2[3],
                         "a4_0": a4[0], "a4_1": a4[1],
                         "a5_0": a5[0], "a5_1": a5[1]}
            shapes = {"a0": [128, LOUT[0] + 3],
                      "t1_1": [128, 2, H[1] + 4], "t2_1": [128, H[1] + 1],
                      "t1_2": [128, 2, H[2] + 4], "t2_2": [128, H[2] + 1],
                      "t1_3": [128, 2, H[3] + 4], "t2_3": [128, H[3] + 1],
                      "a4_0": [128, LOUT[4] + 4], "a4_1": [128, LOUT[4] + 4],
                      "a5_0": [128, LOUT[5] + 4], "a5_1": [128, LOUT[5] + 4]}
            for nm, tt in dbg_tiles.items():
                shp = shapes[nm]
                d = nc.dram_tensor("dbg_" + nm, shp, F,
                                   kind="ExternalOutput")
                dbg_d[nm] = d
                if len(shp) == 3:
                    nc.sync.dma_start(out=d.ap(), in_=tt[:, 0:2, :])
                else:
                    nc.sync.dma_start(out=d.ap(), in_=tt[:, :])

        pspool.release()
        for p in reversed(pools):
            p.release()

    nc.compile()
    _CACHE[("nc", debug)] = nc
    return nc


def kernel(x, w0, b0, w1, b1, w2, b2, w3, b3, w4, b4, w5, b5, w6, b6):
    import os
    from concourse.bass_utils import run_bass_kernel_spmd

    ws = [w0, w1, w2, w3, w4, w5, w6]
    bs = [b0, b1, b2, b3, b4, b5, b6]
    wpk, bpk = _pack_host(ws, bs)
    x = np.asarray(x, np.float32)
    in_maps = [{"xr": _prep_x(x[b]), "wp": wpk, "bp": bpk}
               for b in range(N_CORES)]
    debug = bool(os.environ.get("BITCONV_DEBUG"))
    nc = _build(debug=debug)
    trace = bool(os.environ.get("BITCONV_TRACE"))
    res = run_bass_kernel_spmd(nc, in_maps, core_ids=list(range(N_CORES)),
                               trace=trace)
    if trace or debug:
        if trace:
            print(f"HW exec time: {res.exec_time_ns} ns")
        _CACHE["last_results"] = res
    return np.stack([res.results[b]["y"] for b in range(N_CORES)], axis=0)


# revision 31
# speedup vs baseline: 1.0913x; 1.0182x over previous
"""Trainium2 Bass kernel for a 7-layer ternary-weight (BitNet) 1D conv
feature extractor with exact-erf GELU after each layer.

Contract: kernel(**inputs) takes the FULL inputs from setup_inputs()
(x: [8, 160000] f32, w0..w6 / b0..b6 conv params) and returns the full
output [8, 256, 500] f32.

v2 design (data-parallel, one batch element per core):
- L0 (Cin=1, K=10, stride 5) runs 4x row-tiled (tile_position=(32r,0),
  contraction 10): four 512-col output slices stream concurrently, so a
  2048-col output group costs ~512 PE cycles. Host preps a phase buffer
  xr4[10r+j, 512g+w] = xpad[5*(2048g+512r+w)+j].
- cout=192 layers (L1..L3) store channels 0..127 in a plain tile t1
  [128, 2, H+4] (dim1 = sequence half, one halo col) and channels
  128..191 in a packed tile t2 [128, H+1]: rows 0:64 = first half of
  the sequence, rows 64:128 = second half. t2 is produced with 2x
  column-tiled matmuls (both halves concurrently, M=64 each) and
  consumed with 2x row-tiled K=64 matmuls (both halves concurrently),
  keeping the PE array fully packed where v1 wasted half of it, and
  making every GELU a full 128-partition activation.
- Activations are [128, ~2048] reading a manually rotated 4x1024-col
  PSUM slot ring, amortizing the ~293ns/instr ACT overhead.
- Emission processes each layer's LAST pair first so the halo columns
  (needed by the next layer's HIGH-half reads) exist early; L2 units
  weave into the L0/L1 stream to keep the PE dense (HAM clock at 8/8).
  Known next step (not implemented): split each batch element into two
  independent half-sequence networks per core (host-side halo margins)
  and interleave their units, doubling the effective PSUM pipeline
  depth that currently gates both PE and ACT occupancy.
"""

import numpy as np

# (in_ch, out_ch, kernel, stride, pad) -- fixed problem geometry
LAYERS = [(1, 128, 10, 5, 4), (128, 192, 3, 2, 1), (192, 192, 3, 2, 1),
          (192, 192, 3, 2, 1), (192, 256, 3, 2, 1), (256, 256, 4, 2, 1),
          (256, 256, 4, 2, 1)]
T_IN = 160000
LOUT = [32000, 16000, 8000, 4000, 2000, 1000, 500]
N_CORES = 8
NT = 512          # max matmul free dim / one fp32 PSUM bank
GW = 2048         # PSUM slot width (4 banks)
H = {i: LOUT[i] // 2 for i in (1, 2, 3)}  # packed-tile half lengths


def _slices(n, w=NT):
    return [(o, min(w, n - o)) for o in range(0, n, w)]


def _pairs(i):
    """c1-joint pair supertiles (t0, nst) over [0, H[i])."""
    return _slices(H[i], 1024)


# ---------------- host-side packing ----------------

def _wlayout():
    cols = {}
    tot = 0

    def add(key, n):
        nonlocal tot
        cols[key] = tot
        tot += n

    add((0,), 128)
    add((1, 'c1k128'), 3 * 128)
    add((1, 'c2k128'), 3 * 64)
    for i in (2, 3):
        add((i, 'c1k128'), 3 * 128)
        add((i, 'c1k64'), 3 * 128)
        add((i, 'c2k128'), 3 * 64)
        add((i, 'c2k64'), 3 * 64)
    add((4, 'c1k128'), 3 * 256)   # per tap: [mi=0 | mi=1]
    add((4, 'c1k64'), 3 * 256)
    for i in (5, 6):
        add((i, 'm'), 2 * 4 * 2 * 128)  # ti, k, mi major->minor
    return cols, tot


WCOLS, WTOT = _wlayout()
BCOLS = {}
_nb = 0
for _i in range(7):
    for _mi in range(2 if LAYERS[_i][1] > 128 else 1):
        BCOLS[(_i, _mi)] = _nb
        _nb += 2
NB = _nb


def _pack_host(ws, bs):
    wpk = np.zeros((128, WTOT), np.float16)
    bpk = np.zeros((128, NB), np.float32)
    signs = []
    for i in range(7):
        w = np.asarray(ws[i], np.float32)
        scale = max(float(np.mean(np.abs(w))), 1e-5)
        signs.append(np.clip(np.round(w / scale), -1.0, 1.0))
        b = np.asarray(bs[i], np.float32)
        cout = LAYERS[i][1]
        for mi in range(2 if cout > 128 else 1):
            m0 = 128 * mi
            msz = min(128, cout - m0)
            c = BCOLS[(i, mi)]
            bpk[0:msz, c] = b[m0:m0 + msz]
            bpk[0:msz, c + 1] = scale
            if msz == 64:  # packed chunk: duplicate on rows 64:128
                bpk[64:128, c] = b[m0:m0 + 64]
                bpk[64:128, c + 1] = scale
    f16 = np.float16
    s0 = signs[0][:, 0, :]  # [128, 10]
    for r in range(4):
        wpk[32 * r:32 * r + 10, WCOLS[(0,)]:WCOLS[(0,)] + 128] = \
            s0.T.astype(f16)
    for i in (1, 2, 3, 4):
        s = signs[i]
        cin, cout = LAYERS[i][0], LAYERS[i][1]
        nm1 = 2 if i == 4 else 1
        base = WCOLS[(i, 'c1k128')]
        for k in range(3):
            for mi in range(nm1):
                c0 = base + k * 128 * nm1 + mi * 128
                wpk[0:128, c0:c0 + 128] = \
                    s[128 * mi:128 * mi + 128, 0:128, k].T.astype(f16)
        if cin == 192:
            base = WCOLS[(i, 'c1k64')]
            for k in range(3):
                for mi in range(nm1):
                    c0 = base + k * 128 * nm1 + mi * 128
                    blk = s[128 * mi:128 * mi + 128, 128:192, k].T.astype(f16)
                    wpk[0:64, c0:c0 + 128] = blk
                    wpk[64:128, c0:c0 + 128] = blk
        if cout == 192:
            base = WCOLS[(i, 'c2k128')]
            for k in range(3):
                wpk[0:128, base + 64 * k:base + 64 * k + 64] = \
                    s[128:192, 0:128, k].T.astype(f16)
            if cin == 192:
                base = WCOLS[(i, 'c2k64')]
                for k in range(3):
                    blk = s[128:192, 128:192, k].T.astype(f16)
                    wpk[0:64, base + 64 * k:base + 64 * k + 64] = blk
                    wpk[64:128, base + 64 * k:base + 64 * k + 64] = blk
    for i in (5, 6):
        s = signs[i]
        base = WCOLS[(i, 'm')]
        n = 0
        for ti in range(2):
            for k in range(4):
                for mi in range(2):
                    wpk[0:128, base + n:base + n + 128] = \
                        s[128 * mi:128 * mi + 128,
                          128 * ti:128 * ti + 128, k].T.astype(f16)
                    n += 128
    return wpk, bpk


def _prep_x(xb):
    """xr4 [40, 8192]: xr4[10r+j, 512g+w] = xpad[5*(2048g+512r+w)+j]."""
    xpad = np.zeros(T_IN + 20, np.float16)
    xpad[4:4 + T_IN] = xb.astype(np.float16)
    xr = np.lib.stride_tricks.as_strided(
        xpad, shape=(10, LOUT[0]), strides=(2, 10))
    xr4 = np.zeros((40, 8192), np.float16)
    for g in range(16):
        for r in range(4):
            c0 = 2048 * g + 512 * r
            n = min(512, max(0, LOUT[0] - c0))
            if n:
                xr4[10 * r:10 * r + 10, 512 * g:512 * g + n] = \
                    xr[:, c0:c0 + n]
    return xr4


_CACHE = {}


def _build(debug=False):
    if ("nc", debug) in _CACHE:
        return _CACHE[("nc", debug)]
    from concourse import bacc
    import concourse.mybir as mybir
    import concourse.tile as tile

    F16 = mybir.dt.float16
    F32 = mybir.dt.float32
    GELU = mybir.ActivationFunctionType.Gelu

    nc = bacc.Bacc("TRN2")
    xr_d = nc.dram_tensor("xr", [40, 8192], F16, kind="ExternalInput")
    wp_d = nc.dram_tensor("wp", [128, WTOT], F16, kind="ExternalInput")
    bp_d = nc.dram_tensor("bp", [128, NB], F32, kind="ExternalInput")
    y_d = nc.dram_tensor("y", [256, 500], F32, kind="ExternalOutput")
    dbg_d = {}

    with tile.TileContext(nc) as tc:
        pools = []

        def mkpool(name, bufs=1, space="SBUF"):
            p = tc.alloc_tile_pool(name=name, bufs=bufs, space=space)
            pools.append(p)
            return p

        wpool = mkpool("wpool")
        wt = wpool.tile([128, WTOT], F16, name="wt")
        bt = wpool.tile([128, NB], F32, name="bt")
        scratch = wpool.tile([128, 512], F16, name="scratch")

        apool = mkpool("apool")
        xr4 = apool.tile([128, 8192], F16, name="xr4")
        a0 = apool.tile([128, LOUT[0] + 3], F16, name="a0")
        t1 = {i: apool.tile([128, 2, H[i] + 4], F16, name=f"t1_{i}")
              for i in (1, 2, 3)}
        t2 = {i: apool.tile([128, H[i] + 1], F16, name=f"t2_{i}")
              for i in (1, 2, 3)}
        a4 = [apool.tile([128, LOUT[4] + 4], F16, name=f"a4_{mi}")
              for mi in range(2)]
        a5 = [apool.tile([128, LOUT[5] + 4], F16, name=f"a5_{mi}")
              for mi in range(2)]
        stage = apool.tile([128, 1000], F32, name="stage")

        nc.vector.memset(scratch[:, :], 0.0)
        nc.vector.memset(a0[:, 0:1], 0.0)
        nc.vector.memset(a0[:, LOUT[0] + 1:LOUT[0] + 3], 0.0)
        for i in (1, 2, 3):
            nc.vector.memset(t1[i][:, 0:2, 0:1], 0.0)
            nc.vector.memset(t1[i][:, 0:2, H[i] + 1:H[i] + 3], 0.0)
            nc.vector.memset(t2[i][0:64, 0:1], 0.0)
        for t, L in [(a4[0], LOUT[4]), (a4[1], LOUT[4]),
                     (a5[0], LOUT[5]), (a5[1], LOUT[5])]:
            nc.vector.memset(t[:, 0:1], 0.0)
            nc.vector.memset(t[:, L + 1:L + 3], 0.0)

        pspool = tc.alloc_tile_pool(name="pspool", bufs=1, space="PSUM")
        psall = pspool.tile([128, 4, 1024], F32, name="psall")
        _cur = [0]

        def slots(n):
            """Claim n consecutive 1024-col PSUM slots (round-robin over
            4). Tile's region tracking orders reuse after the prior
            evacuation. Returns the slot index."""
            if _cur[0] % 4 + n > 4:
                _cur[0] += 4 - _cur[0] % 4
            base = _cur[0] % 4
            _cur[0] += n
            return base

        def junk_mms(n):
            for _ in range(n):
                nc.tensor.matmul(psall[:, 0, 0:512], scratch[0:128, 0:128],
                                 scratch[:, :], start=True, stop=True)

        def act(ps_ap, dst_ap, i, mi):
            c = BCOLS[(i, mi)]
            nc.scalar.activation(dst_ap, ps_ap, GELU,
                                 bias=bt[:, c:c + 1],
                                 scale=bt[:, c + 1:c + 2])

        def flush_acts():
            pass

        # ---- L0 unit: output cols [2048g, 2048g+2048) ----
        def u_l0(g):
            sl = slots(2)
            gn = min(GW, LOUT[0] - GW * g)
            for r in range(4):
                n = min(512, max(0, gn - 512 * r))
                if not n:
                    continue
                nc.tensor.matmul(
                    psall[:, sl + r // 2, (r % 2) * 512:(r % 2) * 512 + n],
                    wt[32 * r:32 * r + 10, WCOLS[(0,)]:WCOLS[(0,)] + 128],
                    xr4[32 * r:32 * r + 10, 512 * g:512 * g + n],
                    start=True, stop=True, tile_position=(32 * r, 0))
            if gn == GW:
                act(psall[:, sl:sl + 2, :],
                    a0[:, 1 + GW * g:1 + GW * g + gn], 0, 0)
            else:
                n0 = min(gn, 1024)
                act(psall[:, sl, 0:n0],
                    a0[:, 1 + GW * g:1 + GW * g + n0], 0, 0)
                if gn > 1024:
                    act(psall[:, sl + 1, 0:gn - 1024],
                        a0[:, 1 + GW * g + 1024:1 + GW * g + gn], 0, 0)

        # ---- source AP builders (h = sequence half, local out col t0) --
        def src_l1(h, k, t0, n):
            c = 16000 * h + 2 * t0 + k
            return a0[0:128, c:c + 2 * n:2]

        def src_t1(i):
            def f(h, k, t0, n):
                c = 2 * t0 + k
                return t1[i][0:128, h, c:c + 2 * n:2]
            return f

        # ---- c1-joint unit for layer i in (1,2,3): both halves of the
        # 128-channel chunk for pair p ----
        def u_c1(i, p, src):
            t0, nst = _pairs(i)[p]
            cin = LAYERS[i][0]
            sl = slots(2)
            ps = psall[:, sl:sl + 2, :]
            cols = WCOLS[(i, 'c1k128')]
            for h in range(2):
                for k in range(3):
                    lhsT = wt[0:128, cols + 128 * k:cols + 128 * k + 128]
                    for s0, sn in _slices(nst):
                        nc.tensor.matmul(
                            ps[:, h, s0:s0 + sn], lhsT,
                            src(h, k, t0 + s0, sn),
                            start=k == 0, stop=cin == 128 and k == 2,
                            skip_group_check=True)
            if cin == 192:
                cols64 = WCOLS[(i, 'c1k64')]
                t2p = t2[i - 1]
                for k in range(3):
                    c0 = cols64 + 128 * k
                    for s0, sn in _slices(nst):
                        for h in range(2):
                            nc.tensor.matmul(
                                ps[:, h, s0:s0 + sn],
                                wt[64 * h:64 * h + 64, c0:c0 + 128],
                                t2p[64 * h:64 * h + 64,
                                    2 * (t0 + s0) + k:
                                    2 * (t0 + s0) + k + 2 * sn - 1:2],
                                start=False, stop=k == 2,
                                skip_group_check=True,
                                tile_position=(64 * h, 0))
            act(ps[:, 0:2, 0:nst], t1[i][:, 0:2, 1 + t0:1 + t0 + nst], i, 0)

        # ---- packed c2 unit for layer i in (1,2,3): one pair p ----
        def u_c2(i, p, src):
            t0, nst = _pairs(i)[p]
            cin = LAYERS[i][0]
            sl = slots(1)
            ps = psall[:, sl, :]
            ck = WCOLS[(i, 'c2k128')]
            for k in range(3):
                lhsT = wt[0:128, ck + 64 * k:ck + 64 * k + 64]
                for s0, sn in _slices(nst):
                    for h in range(2):
                        nc.tensor.matmul(
                            ps[64 * h:64 * h + 64, s0:s0 + sn],
                            lhsT, src(h, k, t0 + s0, sn),
                            start=k == 0,
                            stop=cin == 128 and k == 2 and h == 1,
                            skip_group_check=True,
                            tile_position=(0, 64 * h))
            if cin == 192:
                ck64 = WCOLS[(i, 'c2k64')]
                t2p = t2[i - 1]
                for k in range(3):
                    for s0, sn in _slices(nst):
                        for h in range(2):
                            nc.tensor.matmul(
                                ps[64 * h:64 * h + 64, s0:s0 + sn],
                                wt[64 * h:64 * h + 64,
                                   ck64 + 64 * k:ck64 + 64 * k + 64],
                                t2p[64 * h:64 * h + 64,
                                    2 * (t0 + s0) + k:
                                    2 * (t0 + s0) + k + 2 * sn - 1:2],
                                start=False,
                                stop=k == 2 and h == 1,
                                skip_group_check=True,
                                tile_position=(64 * h, 64 * h))
            act(ps[:, 0:nst], t2[i][:, 1 + t0:1 + t0 + nst], i, 1)

        # ---- L4 unit (chunk mi): 2000 out cols, halves of 1000 ----
        def u_l4(mi):
            sl = slots(2)
            ps = psall[:, sl:sl + 2, :]
            cols = WCOLS[(4, 'c1k128')]
            cols64 = WCOLS[(4, 'c1k64')]
            for h in range(2):
                for k in range(3):
                    lhsT = wt[0:128, cols + k * 256 + 128 * mi:
                              cols + k * 256 + 128 * mi + 128]
                    for s0, sn in _slices(1000):
                        nc.tensor.matmul(
                            ps[:, h, s0:s0 + sn], lhsT,
                            src_t1(3)(h, k, s0, sn),
                            start=k == 0, stop=False,
                            skip_group_check=True)
            for k in range(3):
                c0 = cols64 + k * 256 + 128 * mi
                for s0, sn in _slices(1000):
                    for h in range(2):
                        nc.tensor.matmul(
                            ps[:, h, s0:s0 + sn],
                            wt[64 * h:64 * h + 64, c0:c0 + 128],
                            t2[3][64 * h:64 * h + 64,
                                  2 * s0 + k:2 * s0 + k + 2 * sn - 1:2],
                            start=False, stop=k == 2,
                            skip_group_check=True,
                            tile_position=(64 * h, 0))
            act(ps[:, 0, 0:1000], a4[mi][:, 1:1001], 4, mi)
            act(ps[:, 1, 0:1000], a4[mi][:, 1001:2001], 4, mi)

        # ---- L5/L6 plain units ----
        def u_tail(i, mi, srcs, dst):
            lout = LOUT[i]
            sl = slots(1)
            ps = psall[:, sl, :]
            base = WCOLS[(i, 'm')]
            for ti in range(2):
                for k in range(4):
                    c0 = base + (ti * 8 + k * 2 + mi) * 128
                    lhsT = wt[0:128, c0:c0 + 128]
                    for s0, sn in _slices(lout):
                        nc.tensor.matmul(
                            ps[:, s0:s0 + sn], lhsT,
                            srcs[ti][0:128,
                                     2 * s0 + k:2 * s0 + k + 2 * sn:2],
                            start=ti == 0 and k == 0,
                            stop=ti == 1 and k == 3,
                            skip_group_check=True)
            act(ps[:, 0:lout], dst, i, mi)

        # ------------- DMAs -------------
        def dma_xr(lo, hi):
            for r in range(4):
                nc.sync.dma_start(out=xr4[32 * r:32 * r + 10, lo:hi],
                                  in_=xr_d.ap()[10 * r:10 * r + 10, lo:hi])

        dma_xr(0, 2048)
        w_early = WCOLS[(2, 'c1k128')]   # L0+L1 weights
        nc.sync.dma_start(out=wt[:, 0:w_early], in_=wp_d.ap()[:, 0:w_early])
        nc.sync.dma_start(out=bt[:, :], in_=bp_d.ap())
        dma_xr(2048, 8192)
        mid = WCOLS[(5, 'm')]
        nc.gpsimd.dma_start(out=wt[:, w_early:mid],
                            in_=wp_d.ap()[:, w_early:mid])
        nc.gpsimd.dma_start(out=wt[:, mid:WTOT], in_=wp_d.ap()[:, mid:WTOT])

        # ------------- emission -------------
        # L1/L2 pairs are processed with each layer's LAST pair first so
        # the halo columns (needed by the next layer's HIGH-half reads)
        # are available early, letting L2 units weave into the ACT-bound
        # L0/L1 phase to keep the PE dense (HAM clock-gate stays warm).
        junk_mms(14)
        np1 = len(_pairs(1))   # 8 pairs
        for g in (0, 7, 6, 5, 8, 15, 14, 13):
            u_l0(g)
        flush_acts()
        u_c1(1, np1 - 1, src_l1)
        u_c2(1, np1 - 1, src_l1)
        flush_acts()
        nc.vector.tensor_copy(t1[1][:, 1, 0:1], t1[1][:, 0, H[1]:H[1] + 1])
        nc.sync.dma_start(out=t2[1][64:128, 0:1],
                          in_=t2[1][0:64, H[1]:H[1] + 1])
        l0_rest = [1, 9, 2, 10, 3, 11, 4, 12]
        for P in range(np1 - 1):
            if 2 * P + 1 < len(l0_rest):
                u_l0(l0_rest[2 * P])
                u_l0(l0_rest[2 * P + 1])
            u_c1(1, P, src_l1)
            u_c2(1, P, src_l1)
            flush_acts()
            if P >= 3:
                u_c1(2, P - 3, src_t1(1))
            if P >= 5:
                u_c2(2, P - 5, src_t1(1))
        u_c2(2, 2, src_t1(1))
        u_c2(2, 3, src_t1(1))
        flush_acts()
        nc.vector.tensor_copy(t1[2][:, 1, 0:1], t1[2][:, 0, H[2]:H[2] + 1])
        nc.sync.dma_start(out=t2[2][64:128, 0:1],
                          in_=t2[2][0:64, H[2]:H[2] + 1])
        for p in range(2):
            u_c1(3, p, src_t1(2))
            u_c2(3, p, src_t1(2))
            flush_acts()
        nc.vector.tensor_copy(t1[3][:, 1, 0:1], t1[3][:, 0, H[3]:H[3] + 1])
        nc.sync.dma_start(out=t2[3][64:128, 0:1],
                          in_=t2[3][0:64, H[3]:H[3] + 1])
        u_l4(0)
        u_l4(1)
        flush_acts()
        u_tail(5, 0, a4, a5[0][:, 1:1001])
        u_tail(5, 1, a4, a5[1][:, 1:1001])
        flush_acts()
        u_tail(6, 0, a5, stage[:, 0:500])
        flush_acts()
        nc.sync.dma_start(out=y_d.ap()[0:128, :], in_=stage[:, 0:500])
        u_tail(6, 1, a5, stage[:, 500:1000])
        flush_acts()
        nc.sync.dma_start(out=y_d.ap()[128:256, :], in_=stage[:, 500:1000])

        if debug:
            F = mybir.dt.float16
            dbg_tiles = {"a0": a0, "t1_1": t1[1], "t2_1": t2[1],
                         "t1_2": t1[2], "t2_2": t2[2],
                         "t1_3": t1[3], "t2_3": t2[3],
                         "a4_0": a4[0], "a4_1": a4[1],
                         "a5_0": a5[0], "a5_1": a5[1]}
            shapes = {"a0": [128, LOUT[0] + 3],
                      "t1_1": [128, 2, H[1] + 4], "t2_1": [128, H[1] + 1],
                      "t1_2": [128, 2, H[2] + 4], "t2_2": [128, H[2] + 1],
                      "t1_3": [128, 2, H[3] + 4], "t2_3": [128, H[3] + 1],
                      "a4_0": [128, LOUT[4] + 4], "a4_1": [128, LOUT[4] + 4],
                      "a5_0": [128, LOUT[5] + 4], "a5_1": [128, LOUT[5] + 4]}
            for nm, tt in dbg_tiles.items():
                shp = shapes[nm]
                d = nc.dram_tensor("dbg_" + nm, shp, F,
                                   kind="ExternalOutput")
                dbg_d[nm] = d
                if len(shp) == 3:
                    nc.sync.dma_start(out=d.ap(), in_=tt[:, 0:2, :])
                else:
                    nc.sync.dma_start(out=d.ap(), in_=tt[:, :])

        pspool.release()
        for p in reversed(pools):
            p.release()

    nc.compile()
    _CACHE[("nc", debug)] = nc
    return nc


def kernel(x, w0, b0, w1, b1, w2, b2, w3, b3, w4, b4, w5, b5, w6, b6):
    import os
    from concourse.bass_utils import run_bass_kernel_spmd

    ws = [w0, w1, w2, w3, w4, w5, w6]
    bs = [b0, b1, b2, b3, b4, b5, b6]
    wpk, bpk = _pack_host(ws, bs)
    x = np.asarray(x, np.float32)
    in_maps = [{"xr": _prep_x(x[b]), "wp": wpk, "bp": bpk}
               for b in range(N_CORES)]
    debug = bool(os.environ.get("BITCONV_DEBUG"))
    nc = _build(debug=debug)
    trace = bool(os.environ.get("BITCONV_TRACE"))
    res = run_bass_kernel_spmd(nc, in_maps, core_ids=list(range(N_CORES)),
                               trace=trace)
    if trace or debug:
        if trace:
            print(f"HW exec time: {res.exec_time_ns} ns")
        _CACHE["last_results"] = res
    return np.stack([res.results[b]["y"] for b in range(N_CORES)], axis=0)
